# revision 1
# baseline (speedup 1.0000x reference)
"""Trainium2 Bass kernel for nn_CUFLayer_83640193122985.

CUF layer: per-pixel hypernet MLP (118->32->32->32->32->2304) generates 3x3
per-channel kernels at each of 128x128 target pixels; applied to the 2x
nearest-upsampled main_input [4,64,64,256]; then 1x1 projection [256->128].

Key algebraic optimization (parity decomposition): the upsample is exactly 2x
nearest-neighbor, so each output pixel's 3x3 window covers only 2x2 DISTINCT
source pixels; which taps collapse onto which source pixel depends only on the
output pixel's (row, col) parity. W_out/b_out columns are pre-combined on the
host per parity class, turning 9 multiply-taps into 4 and letting the whole
apply stage run at source resolution.

Sharding: 8-way data parallel over output rows (16 rows/core, all batches),
hypernet recomputed per-core for its slab; no collectives. The DCT feature
matrix is input-independent and precomputed on host. Matmuls run in float32r
(near-fp32 precision at full PE rate); the per-pixel multiply runs in bf16 on
the vector engine (2x packed mode, batch-broadcast); tap and channel
accumulation ride the PE's PSUM accumulation fused with the 1x1 projection.
Inputs are packed into few DRAM tensors in need-order (HWDGE dispatch is
~0.65us/DMA, transfers serialize at ~360GB/s); per-class outputs accumulate in
two 2-bank PSUM batch-pair tiles whose copy+DMA overlap each other's matmuls.

Self-contained: hardcodes all shapes; no sibling imports.
"""

import numpy as np
import ml_dtypes

import concourse.bass as bass
import concourse.mybir as mybir
import concourse.tile as tile
from concourse import bacc
from concourse import bass_utils

BF16 = ml_dtypes.bfloat16
F32R = mybir.dt.float32r

K = 3
DCT_BASIS = 25
B, H_IN, W_IN, C = 4, 64, 64, 256
H_T, W_T, F_OUT = 128, 128, 128
N_CORES = 8
RPC = H_T // N_CORES  # 16 output rows per core
D_IN = 118
NPIX = RPC * W_T  # 2048 pixels per core
MROWS = RPC // 2 + 2  # 10 source rows incl halo
MCOLS = W_IN + 2  # 66 source cols incl halo
QR = RPC // 2  # 8 source-row positions per core
QC = W_IN  # 64 source-col positions

# vertical tap-collapse table: V[pi][ai] = (alpha, [di...]); same for cols
_V = {0: [(-1, [0]), (0, [1, 2])], 1: [(0, [0, 1]), (1, [2])]}
_CLASSES = [(0, 0), (0, 1), (1, 0), (1, 1)]

_CACHE: dict = {}


# ----------------------------------------------------------------- host side
def _build_features():
    """feat [H_T, W_T, 118] fp32 — input-independent constant."""
    f = np.linspace(1.0, 2.0, DCT_BASIS).astype(np.float32)
    gh = np.linspace(0.0, 1.0, H_T).astype(np.float32)
    row_enc = np.cos(np.pi * (2.0 * gh[:, None] + 1.0) * f[None, :]).astype(np.float32)
    delta = np.concatenate(
        [
            np.broadcast_to(row_enc[:, None, :], (H_T, W_T, DCT_BASIS)),
            np.broadcast_to(row_enc[None, :, :], (H_T, W_T, DCT_BASIS)),
        ],
        axis=-1,
    )
    scale = np.array([H_T / H_IN, W_T / W_IN], np.float32)
    scale_enc = np.cos(np.pi * (2.0 * scale[:, None] + 1.0) * f[None, :]).reshape(-1)
    offs = np.arange(K, dtype=np.float32) - 1.0
    ki, kj = np.meshgrid(offs, offs, indexing="ij")
    kidx = np.stack([ki, kj], -1).reshape(K * K, 2)
    f9 = np.linspace(1.0, 1.0, 9).astype(np.float32)
    kenc = np.cos(np.pi * (2.0 * kidx[..., None] + 1.0) * f9).reshape(K * K, 18).mean(0)
    feat = np.concatenate(
        [
            delta,
            np.broadcast_to(scale_enc, (H_T, W_T, 50)),
            np.broadcast_to(kenc.astype(np.float32), (H_T, W_T, 18)),
        ],
        axis=-1,
    ).astype(np.float32)
    return feat  # [128,128,118]


def _chunk_meta():
    """Per combined-kernel chunk m = class*8 + A*2 + cc: (class, pi, pj,
    alpha, beta, cc, taps). A = ai*2 + bi."""
    meta = []
    for ci, (pi, pj) in enumerate(_CLASSES):
        for ai in range(2):
            for bi in range(2):
                alpha, dis = _V[pi][ai]
                beta, djs = _V[pj][bi]
                taps = [di * 3 + dj for di in dis for dj in djs]
                for cc in range(2):
                    meta.append((ci, pi, pj, alpha, beta, cc, taps))
    return meta


def _host_prep(inputs):
    """Build per-core input maps (few, large tensors to minimize DMA count)."""
    main_input = np.asarray(inputs["main_input"], np.float32)
    feat = _CACHE.get("feat")
    if feat is None:
        feat = _CACHE["feat"] = _build_features()

    # source image, zero-padded by 1: [B, 66, 66, C] then channel-major bf16
    mp = np.pad(main_input, ((0, 0), (1, 1), (1, 1), (0, 0)))

    Wout = np.asarray(inputs["W_out"], np.float32)  # [32, 2304] cols t*256+c
    bout = np.asarray(inputs["b_out"], np.float32)
    wcomb = np.empty((32, 32 * 128), np.float32)
    bcomb = np.empty((128, 32), np.float32)
    for m, (ci, pi, pj, al, be, cc, taps) in enumerate(_chunk_meta()):
        Wc = sum(Wout[:, t * 256 + cc * 128 : t * 256 + (cc + 1) * 128] for t in taps)
        bc = sum(bout[t * 256 + cc * 128 : t * 256 + (cc + 1) * 128] for t in taps)
        wcomb[:, m * 128 : (m + 1) * 128] = Wc
        bcomb[:, m] = bc

    # wm: w2 | w3 | w4ext | wcomb_ext  -> [33, 97 + 4096]; row 32 carries bcomb
    # (the kern matmul consumes an appended ones-row in h4, folding the bias
    # into the PE accumulation at zero cost)
    wm = np.zeros((33, 97 + 32 * 128), np.float32)
    wm[:32, 0:32] = np.asarray(inputs["W2"], np.float32)
    wm[:32, 32:64] = np.asarray(inputs["W3"], np.float32)
    # W4 gets a 33rd output column of zeros; with bias 1.0 it yields the
    # constant ones-row in h4 that carries bcomb through the kern matmul
    wm[:32, 64:96] = np.asarray(inputs["W4"], np.float32)
    wm[:32, 97:] = wcomb
    wm[32, 97:] = bcomb.T.reshape(-1)
    # bs: b1..b4 -> [33, 4]; bs[32, 3] = 1.0 feeds the h4 ones-row
    bs = np.zeros((33, 4), np.float32)
    for i in (1, 2, 3, 4):
        bs[:32, i - 1] = np.asarray(inputs[f"b{i}"], np.float32)
    bs[32, 3] = 1.0
    bb = np.asarray(inputs["b_proj"], np.float32).reshape(128, 1)
    wproj = np.ascontiguousarray(
        np.asarray(inputs["W_proj"], np.float32).reshape(2, 128, F_OUT).transpose(1, 0, 2)
    ).astype(BF16)  # [128c, 2cc, F]

    w1 = np.asarray(inputs["W1"], np.float32)  # [118, 32]
    in_maps = []
    for k in range(N_CORES):
        m0 = k * QR  # first source row of this core's slab
        slab = mp[:, m0 : m0 + MROWS, :, :]  # [B,10,66,C]
        x_cm = np.ascontiguousarray(slab.transpose(3, 0, 1, 2)).reshape(
            2, 128, B, MROWS, MCOLS
        ).astype(BF16)
        # feature columns grouped by parity class: (class, q, j); append W1
        r0 = k * RPC
        fs = feat[r0 : r0 + RPC]  # [16,128,118]
        fcls = np.stack(
            [fs[pi::2, pj::2].reshape(QR * QC, D_IN) for (pi, pj) in _CLASSES]
        )  # [4, 512, 118]
        fw1 = np.concatenate(
            [w1, np.ascontiguousarray(fcls.reshape(4 * QR * QC, D_IN).T)], axis=1
        )  # [118, 2080] = [w1 | feat]
        in_maps.append({"x": x_cm, "fw1": fw1, "wm": wm, "bs": bs, "bb": bb,
                        "wproj": wproj})
    return in_maps


def _gather(results):
    """results[k]["y"] [F, 4class, B, 512] bf16 -> [B, H_T, W_T, F] fp32."""
    out = np.empty((B, H_T, W_T, F_OUT), np.float32)
    for k, res in enumerate(results):
        y5 = np.asarray(res["y"]).astype(np.float32).reshape(F_OUT, 4, B, QR, QC)
        slab = out[:, k * RPC : (k + 1) * RPC]  # [B,16,128,F] view
        for ci, (pi, pj) in enumerate(_CLASSES):
            slab[:, pi::2, pj::2] = y5[:, ci].transpose(1, 2, 3, 0)
    return out


# -------------------------------------------------------------- device program
def _build_program(repeat: int = 1, loop_repeat: int = 1, staggered: bool = False):
    f32, bf16 = mybir.dt.float32, mybir.dt.bfloat16
    Relu = mybir.ActivationFunctionType.Relu
    Ident = mybir.ActivationFunctionType.Identity

    nc = bacc.Bacc("TRN2", target_bir_lowering=False, debug=False, num_devices=N_CORES)
    x_d = nc.dram_tensor("x", (2, 128, B, MROWS, MCOLS), bf16, kind="ExternalInput")
    fw1_d = nc.dram_tensor("fw1", (D_IN, NPIX + 32), F32R, kind="ExternalInput")
    wm_d = nc.dram_tensor("wm", (33, 97 + 32 * 128), F32R, kind="ExternalInput")
    bs_d = nc.dram_tensor("bs", (33, 4), f32, kind="ExternalInput")
    bb_d = nc.dram_tensor("bb", (128, 1), f32, kind="ExternalInput")
    wproj_d = nc.dram_tensor("wproj", (128, 2, F_OUT), bf16, kind="ExternalInput")
    y_d = nc.dram_tensor("y", (F_OUT, 4, B, 512), bf16, kind="ExternalOutput")

    meta = _chunk_meta()

    with tile.TileContext(nc) as tc:
        with (
            tc.tile_pool(name="const", bufs=1) as const,
            tc.tile_pool(name="hbuf", bufs=2) as hbuf,
            tc.tile_pool(name="kern", bufs=2) as kern_pool,
            tc.tile_pool(name="zbuf", bufs=8) as zbuf,
            tc.tile_pool(name="ybuf", bufs=2) as ybuf,
            tc.tile_pool(name="ps_mlp", bufs=2, space="PSUM") as ps_mlp,
            tc.tile_pool(name="ps_kern", bufs=1, space="PSUM") as ps_kern,
            tc.tile_pool(name="ps_y", bufs=1, space="PSUM") as ps_y,
        ):
            # ---- input loads, in need-order: MLP biases + w1 + class-0
            # features, MLP/kern weights, the image, remaining features,
            # projection weights ----
            bs_sb = const.tile([33, 4], f32)
            nc.sync.dma_start(bs_sb, bs_d[:])
            fw1_sb = const.tile([D_IN, NPIX + 32], F32R)
            nc.sync.dma_start(fw1_sb[:, 0:544], fw1_d[:, 0:544])
            wm_sb = const.tile([33, 97 + 32 * 128], F32R)
            nc.sync.dma_start(wm_sb, wm_d[:])
            x_sb = const.tile([128, 2, B, MROWS, MCOLS], bf16)
            nc.sync.dma_start(x_sb, x_d[:].transpose((1, 0, 2, 3, 4)))
            nc.sync.dma_start(fw1_sb[:, 544:], fw1_d[:, 544:])
            wproj_sb = const.tile([128, 2, F_OUT], bf16)
            nc.sync.dma_start(wproj_sb, wproj_d[:])
            bb_sb = const.tile([128, 1], f32)
            nc.sync.dma_start(bb_sb, bb_d[:])

            w_sb = {
                1: fw1_sb[:, 0:32],
                2: wm_sb[0:32, 0:32],
                3: wm_sb[0:32, 32:64],
                4: wm_sb[0:32, 64:97],
            }
            wcomb_sb = wm_sb[:, 97 : 97 + 32 * 128]

            def _body_all():
                reps = [c for _ in range(repeat) for c in range(4)]

                def mlp(ci):
                    h = fw1_sb[:, 32 + ci * 512 : 32 + (ci + 1) * 512]
                    if "hyper" in SKIP:
                        return hbuf.tile([33, 512], F32R, tag=f"h{ci}", name="hn")
                    for i in range(1, 5):
                        rows = 33 if i == 4 else 32
                        ps = ps_mlp.tile([rows, 512], f32, tag="mlp", name="ps")
                        nc.tensor.matmul(ps, w_sb[i], h, start=True, stop=True)
                        hn = hbuf.tile([rows, 512], F32R, tag=f"h{ci}", name="hn")
                        nc.scalar.activation(
                            hn, ps, Relu, bias=bs_sb[0:rows, i - 1 : i], scale=1.0
                        )
                        h = hn
                    return h

                def kerns(ci, h):
                    # one [128,1024] 2-bank psum + one ACT copy per A-pair
                    # (both c-halves share alpha/beta)
                    tiles = {}
                    for A in range(4):
                        m0 = ci * 8 + A * 2
                        _, _, _, alpha, beta, _, _ = meta[m0]
                        if "hyper" in SKIP:
                            wide = MCOLS if beta == 0 else QC
                            km = kern_pool.tile(
                                [128, 2, QR, wide], bf16, tag=f"k{A}", name="km"
                            )
                            tiles[A] = (km, alpha, beta)
                            continue
                        ps = ps_kern.tile([128, 1024], f32, tag="kern_ps", name="ps")
                        for cc in range(2):
                            nc.tensor.matmul(
                                ps[:, cc * 512 : (cc + 1) * 512],
                                wcomb_sb[:, (m0 + cc) * 128 : (m0 + cc + 1) * 128],
                                h,
                                start=True,
                                stop=True,
                            )
                        ps28 = ps.rearrange("p (c a b) -> p c a b", c=2, a=QR)
                        wide = MCOLS if beta == 0 else QC
                        km = kern_pool.tile(
                            [128, 2, QR, wide], bf16, tag=f"k{A}", name="km"
                        )
                        if beta == 0:
                            border = bass.AP(
                                tensor=km.tensor,
                                offset=km.offset,
                                ap=[km.ap[0], km.ap[1], km.ap[2], [65, 2]],
                            )
                            nc.gpsimd.memset(border, 0.0)
                            for _ in range(2 if "kcopy" in DUP else 1):
                                nc.scalar.copy(km[:, :, :, 1:65], ps28)
                        else:
                            for _ in range(2 if "kcopy" in DUP else 1):
                                nc.scalar.copy(km, ps28)
                        tiles[A] = (km, alpha, beta)
                    return tiles

                def apply(ci, kern_tiles):
                    # one batch-broadcast multiply per A-chunk covering both
                    # channel halves (same row/col shift), feeding two 2-bank
                    # PSUM accumulators (batch pairs); the first pair's
                    # copy+DMA overlaps the second pair's matmul pass
                    za = {}
                    for A in range(4):
                        km, alpha, beta = kern_tiles[A]
                        kb = bass.AP(
                            tensor=km.tensor,
                            offset=km.offset,
                            ap=[km.ap[0], km.ap[1], [0, B], *km.ap[2:]],
                        )
                        rows = slice(1 + alpha, 1 + alpha + QR)
                        if beta == 0:
                            z = zbuf.tile([128, 2, B, QR, MCOLS], bf16,
                                          tag="z", name="z")
                            for _ in range(2 if "prod" in DUP else 1):
                                nc.vector.tensor_mul(
                                    z, x_sb[:, :, :, rows, 0:MCOLS], kb
                                )
                            za[A] = [
                                [z[:, cc, b, :, 1:65] for b in range(B)]
                                for cc in range(2)
                            ]
                        else:
                            c0 = 1 + beta  # 0 or 2, 4B-aligned either way
                            z = zbuf.tile([128, 2, B, QR, QC], bf16,
                                          tag="z", name="z")
                            for _ in range(2 if "prod" in DUP else 1):
                                nc.vector.tensor_mul(
                                    z, x_sb[:, :, :, rows, c0 : c0 + QC], kb
                                )
                            za[A] = [
                                [z[:, cc, b] for b in range(B)]
                                for cc in range(2)
                            ]
                    zs = [za[A][cc] for cc in range(2) for A in range(4)]
                    for half in range(2) if "apply" not in SKIP else []:
                        yp = ps_y.tile(
                            [128, 1024], f32, tag=f"y{half}", name="yp"
                        )
                        for rep_mm in range(2 if "ymm" in DUP else 1):
                            for i, rhss in enumerate(zs):
                                for b2 in range(2):
                                    nc.tensor.matmul(
                                        yp[:, b2 * 512 : (b2 + 1) * 512].rearrange(
                                            "p (a b) -> p a b", a=QR
                                        ),
                                        wproj_sb[:, (i // 4) % 2, :],
                                        rhss[half * 2 + b2],
                                        start=(i == 0),
                                        stop=(i == 7),
                                    )
                        ys = ybuf.tile([F_OUT, 1024], bf16, tag="ysb", name="ys")
                        nc.scalar.activation(
                            ys, yp, Ident, bias=bb_sb[:, 0:1], scale=1.0
                        )
                        nc.sync.dma_start(
                            y_d[:, ci, half * 2 : half * 2 + 2],
                            ys.rearrange("p (a b) -> p a b", a=2),
                        )

                # software pipeline: all MLPs upfront (class-major), kern
                # production one class ahead of its apply stage
                hs = [mlp(ci) for ci in reps]
                ks = kerns(reps[0], hs[0])
                for idx, ci in enumerate(reps):
                    ks_next = (
                        kerns(reps[idx + 1], hs[idx + 1])
                        if idx + 1 < len(reps) else None
                    )
                    apply(ci, ks)
                    ks = ks_next

            if loop_repeat > 1:
                with tc.For_i(
                    0, loop_repeat, 1,
                    hint_engines=(mybir.EngineType.PE, mybir.EngineType.Activation),
                    staggered_reset=staggered,
                ):
                    _body_all()
            else:
                _body_all()

    nc.compile()
    return nc


import os

UNROLL = int(os.environ.get("CUF_UNROLL", "4"))
STAGGERED = bool(int(os.environ.get("CUF_STAG", "0")))
# timing-ablation knobs (local experiments only)
SKIP = set(os.environ.get("CUF_SKIP", "").split(","))
DUP = set(os.environ.get("CUF_DUP", "").split(","))


def get_program(repeat: int = 1, loop_repeat: int = 1):
    # Amortize the For_i back-edge barrier (~2us drain + sem reset) and the
    # per-iteration pipeline fill/drain by unrolling the body inside the
    # hardware loop whenever the requested trip count allows it.
    if repeat == 1 and loop_repeat > 1 and loop_repeat % UNROLL == 0:
        repeat, loop_repeat = UNROLL, loop_repeat // UNROLL
    key = f"nc{repeat}_{loop_repeat}"
    nc = _CACHE.get(key)
    if nc is None:
        nc = _CACHE[key] = _build_program(repeat, loop_repeat, STAGGERED)
    return nc


# --------------------------------------------------------------------- entry
def kernel(**inputs) -> np.ndarray:
    nc = get_program()
    in_maps = _host_prep(inputs)
    res = bass_utils.run_bass_kernel_spmd(
        nc, in_maps, core_ids=list(range(N_CORES))
    )
    return _gather(res.results)



# revision 16
# speedup vs baseline: 1.0985x; 1.0985x over previous
"""Trainium2 Bass kernel for nn_CUFLayer_83640193122985.

CUF layer: per-pixel hypernet MLP (118->32->32->32->32->2304) generates 3x3
per-channel kernels at each of 128x128 target pixels; applied to the 2x
nearest-upsampled main_input [4,64,64,256]; then 1x1 projection [256->128].

Key algebraic optimization (parity decomposition): the upsample is exactly 2x
nearest-neighbor, so each output pixel's 3x3 window covers only 2x2 DISTINCT
source pixels; which taps collapse onto which source pixel depends only on the
output pixel's (row, col) parity. W_out/b_out columns are pre-combined on the
host per parity class, turning 9 multiply-taps into 4 and letting the whole
apply stage run at source resolution.

v2 structure (engine-balanced around the DVE product floor):
- MLP col-tiled: layer 1 runs as 4 col-group matmuls into one [128,512] PSUM
  bank so all 4 parity classes' hidden states stack on partition groups;
  layers 2-4 are single [128,128] block-diagonal matmuls. 4 relus total.
- Kern matmuls drop the ones-row (K=32) and run row-tiled: one round loads a
  [128,128] stacked weight block (4 classes x one chunk) and issues 4
  concurrent sub-array matmuls into 4 PSUM banks. The kern bias rides the
  PSUM->SBUF ACT copy's per-partition bias operand.
- Apply keeps PSUM accumulation for the tap-collapse sum (PE pass costs less
  than a DVE add at tensor_tensor's 2x ceiling); per-pixel products are the
  DVE bottleneck (~35us/core) that everything else hides under.

Sharding: 8-way data parallel over output rows (16 rows/core, all batches),
hypernet recomputed per-core for its slab; no collectives.

Self-contained: hardcodes all shapes; no sibling imports.
"""

import numpy as np
import ml_dtypes

import concourse.bass as bass
import concourse.mybir as mybir
import concourse.tile as tile
from concourse import bacc
from concourse import bass_utils

BF16 = ml_dtypes.bfloat16
F32R = mybir.dt.float32r

K = 3
DCT_BASIS = 25
B, H_IN, W_IN, C = 4, 64, 64, 256
H_T, W_T, F_OUT = 128, 128, 128
N_CORES = 8
RPC = H_T // N_CORES  # 16 output rows per core
D_IN = 118
NPIX = RPC * W_T  # 2048 pixels per core
MROWS = RPC // 2 + 2  # 10 source rows incl halo
MCOLS = W_IN + 2  # 66 source cols incl halo
QR = RPC // 2  # 8 source-row positions per core
QC = W_IN  # 64 source-col positions

# vertical tap-collapse table: V[pi][ai] = (alpha, [di...]); same for cols
_V = {0: [(-1, [0]), (0, [1, 2])], 1: [(0, [0, 1]), (1, [2])]}
_CLASSES = [(0, 0), (0, 1), (1, 0), (1, 1)]

_CACHE: dict = {}


# ----------------------------------------------------------------- host side
def _build_features():
    """feat [H_T, W_T, 118] fp32 — input-independent constant."""
    f = np.linspace(1.0, 2.0, DCT_BASIS).astype(np.float32)
    gh = np.linspace(0.0, 1.0, H_T).astype(np.float32)
    row_enc = np.cos(np.pi * (2.0 * gh[:, None] + 1.0) * f[None, :]).astype(np.float32)
    delta = np.concatenate(
        [
            np.broadcast_to(row_enc[:, None, :], (H_T, W_T, DCT_BASIS)),
            np.broadcast_to(row_enc[None, :, :], (H_T, W_T, DCT_BASIS)),
        ],
        axis=-1,
    )
    scale = np.array([H_T / H_IN, W_T / W_IN], np.float32)
    scale_enc = np.cos(np.pi * (2.0 * scale[:, None] + 1.0) * f[None, :]).reshape(-1)
    offs = np.arange(K, dtype=np.float32) - 1.0
    ki, kj = np.meshgrid(offs, offs, indexing="ij")
    kidx = np.stack([ki, kj], -1).reshape(K * K, 2)
    f9 = np.linspace(1.0, 1.0, 9).astype(np.float32)
    kenc = np.cos(np.pi * (2.0 * kidx[..., None] + 1.0) * f9).reshape(K * K, 18).mean(0)
    feat = np.concatenate(
        [
            delta,
            np.broadcast_to(scale_enc, (H_T, W_T, 50)),
            np.broadcast_to(kenc.astype(np.float32), (H_T, W_T, 18)),
        ],
        axis=-1,
    ).astype(np.float32)
    return feat  # [128,128,118]


def _chunk_meta():
    """Per class g, A = ai*2+bi: (alpha, beta, taps)."""
    meta = []
    for g, (pi, pj) in enumerate(_CLASSES):
        per_a = []
        for ai in range(2):
            for bi in range(2):
                alpha, dis = _V[pi][ai]
                beta, djs = _V[pj][bi]
                taps = [di * 3 + dj for di in dis for dj in djs]
                per_a.append((alpha, beta, taps))
        meta.append(per_a)
    return meta  # [4 classes][4 A] -> (alpha, beta, taps)


def _host_prep(inputs):
    """Build per-core input maps (few, large tensors to minimize DMA count)."""
    main_input = np.asarray(inputs["main_input"], np.float32)
    feat = _CACHE.get("feat")
    if feat is None:
        feat = _CACHE["feat"] = _build_features()

    # source image, zero-padded by 1: [B, 66, 66, C] then channel-major bf16
    mp = np.pad(main_input, ((0, 0), (1, 1), (1, 1), (0, 0)))

    Wout = np.asarray(inputs["W_out"], np.float32)  # [32, 2304] cols t*256+c
    bout = np.asarray(inputs["b_out"], np.float32)
    meta = _chunk_meta()
    # stacked kern weights: rows 32g..32g+31 = class g; col block m=A*2+cc
    wstk = np.zeros((128, 8 * 128), np.float32)
    bcomb = np.zeros((4, 4, 2, 128), np.float32)  # [g, A, cc, c]
    for g in range(4):
        for A in range(4):
            _, _, taps = meta[g][A]
            for cc in range(2):
                Wc = sum(Wout[:, t * 256 + cc * 128: t * 256 + (cc + 1) * 128]
                         for t in taps)
                bc = sum(bout[t * 256 + cc * 128: t * 256 + (cc + 1) * 128]
                         for t in taps)
                m = A * 2 + cc
                wstk[32 * g: 32 * g + 32, m * 128: (m + 1) * 128] = Wc
                bcomb[g, A, cc] = bc
    bk = np.ascontiguousarray(
        bcomb.transpose(3, 0, 1, 2).reshape(128, 32))  # [c, g*8+A*2+cc]

    # block-diagonal W2..W4 [128, 3*128]
    wblk = np.zeros((128, 3 * 128), np.float32)
    for i in (2, 3, 4):
        Wi = np.asarray(inputs[f"W{i}"], np.float32)
        for g in range(4):
            wblk[32 * g: 32 * g + 32,
                 (i - 2) * 128 + 32 * g: (i - 2) * 128 + 32 * g + 32] = Wi

    # relu biases, replicated per class group: [128, 4]
    bs = np.zeros((128, 4), np.float32)
    for i in (1, 2, 3, 4):
        bi = np.asarray(inputs[f"b{i}"], np.float32)
        for g in range(4):
            bs[32 * g: 32 * g + 32, i - 1] = bi

    bb = np.asarray(inputs["b_proj"], np.float32).reshape(128, 1)
    wproj = np.ascontiguousarray(
        np.asarray(inputs["W_proj"], np.float32).reshape(2, 128, F_OUT).transpose(1, 0, 2)
    ).astype(BF16)  # [128c, 2cc, F]

    w1 = np.asarray(inputs["W1"], np.float32)  # [118, 32]
    in_maps = []
    for k in range(N_CORES):
        m0 = k * QR  # first source row of this core's slab
        slab = mp[:, m0: m0 + MROWS, :, :]  # [B,10,66,C]
        x_cm = np.ascontiguousarray(slab.transpose(3, 0, 1, 2)).reshape(
            2, 128, B, MROWS, MCOLS
        ).astype(BF16)
        # feature columns grouped by parity class: (class, q, j); append W1
        r0 = k * RPC
        fs = feat[r0: r0 + RPC]  # [16,128,118]
        fcls = np.stack(
            [fs[pi::2, pj::2].reshape(QR * QC, D_IN) for (pi, pj) in _CLASSES]
        )  # [4, 512, 118]
        # layer-1 weights as 4 block-column copies: W1blk_g [128, 128] has W1
        # in cols 32g..32g+31, zero elsewhere. Accumulating the 4 per-class
        # matmuls in one PSUM tile assembles h1 in blocked [4*32, 512] layout
        # without any PE tile-positioning.
        w1blk = np.zeros((128, 4 * 128), np.float32)
        for g in range(4):
            w1blk[:D_IN, g * 128 + 32 * g: g * 128 + 32 * g + 32] = w1
        featT = np.zeros((128, 4 * QR * QC), np.float32)
        featT[:D_IN] = fcls.reshape(4 * QR * QC, D_IN).T
        fw1 = np.concatenate([w1blk, featT], axis=1)  # [128, 512 + 2048]
        in_maps.append({"x": x_cm, "fw1": fw1, "wblk": wblk, "wstk": wstk,
                        "bs": bs, "bk": bk, "bb": bb, "wproj": wproj})
    return in_maps


def _gather(results):
    """results[k]["y"] [F, 4class, B, 512] bf16 -> [B, H_T, W_T, F] fp32."""
    out = np.empty((B, H_T, W_T, F_OUT), np.float32)
    for k, res in enumerate(results):
        y5 = np.asarray(res["y"]).astype(np.float32).reshape(F_OUT, 4, B, QR, QC)
        slab = out[:, k * RPC: (k + 1) * RPC]  # [B,16,128,F] view
        for ci, (pi, pj) in enumerate(_CLASSES):
            slab[:, pi::2, pj::2] = y5[:, ci].transpose(1, 2, 3, 0)
    return out


# -------------------------------------------------------------- device program
def _build_program(repeat: int = 1, loop_repeat: int = 1, staggered: bool = False):
    f32, bf16 = mybir.dt.float32, mybir.dt.bfloat16
    Relu = mybir.ActivationFunctionType.Relu
    Ident = mybir.ActivationFunctionType.Identity

    nc = bacc.Bacc("TRN2", target_bir_lowering=False, debug=False, num_devices=N_CORES)
    x_d = nc.dram_tensor("x", (2, 128, B, MROWS, MCOLS), bf16, kind="ExternalInput")
    fw1_d = nc.dram_tensor("fw1", (128, NPIX + 512), F32R, kind="ExternalInput")
    wblk_d = nc.dram_tensor("wblk", (128, 3 * 128), F32R, kind="ExternalInput")
    wstk_d = nc.dram_tensor("wstk", (128, 8 * 128), F32R, kind="ExternalInput")
    bs_d = nc.dram_tensor("bs", (128, 4), f32, kind="ExternalInput")
    bk_d = nc.dram_tensor("bk", (128, 32), f32, kind="ExternalInput")
    bb_d = nc.dram_tensor("bb", (128, 1), f32, kind="ExternalInput")
    wproj_d = nc.dram_tensor("wproj", (128, 2, F_OUT), bf16, kind="ExternalInput")
    y_d = nc.dram_tensor("y", (F_OUT, 4, B, 512), bf16, kind="ExternalOutput")

    meta = _chunk_meta()

    with tile.TileContext(nc) as tc:
        with (
            tc.tile_pool(name="const", bufs=1) as const,
            tc.tile_pool(name="hbuf", bufs=2) as hbuf,
            tc.tile_pool(name="kern", bufs=2) as kern_pool,
            tc.tile_pool(name="zbuf", bufs=8) as zbuf,
            tc.tile_pool(name="ybuf", bufs=2) as ybuf,
            tc.tile_pool(name="ps_mlp", bufs=1, space="PSUM") as ps_mlp,
            tc.tile_pool(name="ps_kern", bufs=1, space="PSUM") as ps_kern,
            tc.tile_pool(name="ps_y", bufs=1, space="PSUM") as ps_y,
        ):
            # ---- input loads, in need-order ----
            bs_sb = const.tile([128, 4], f32)
            nc.sync.dma_start(bs_sb, bs_d[:])
            fw1_sb = const.tile([128, NPIX + 512], F32R)
            nc.sync.dma_start(fw1_sb, fw1_d[:])
            wblk_sb = const.tile([128, 3 * 128], F32R)
            nc.sync.dma_start(wblk_sb, wblk_d[:])
            wstk_sb = const.tile([128, 8 * 128], F32R)
            nc.sync.dma_start(wstk_sb, wstk_d[:])
            bk_sb = const.tile([128, 32], f32)
            nc.sync.dma_start(bk_sb, bk_d[:])
            x_sb = const.tile([128, 2, B, MROWS, MCOLS], bf16)
            nc.sync.dma_start(x_sb, x_d[:].transpose((1, 0, 2, 3, 4)))
            wproj_sb = const.tile([128, 2, F_OUT], bf16)
            nc.sync.dma_start(wproj_sb, wproj_d[:])
            bb_sb = const.tile([128, 1], f32)
            nc.sync.dma_start(bb_sb, bb_d[:])

            def _body_all():
                reps = list(range(repeat))

                def mlp(r):
                    # all 4 classes at once: h tiles [128, 512] f32r, class g
                    # on partitions 32g..32g+31
                    if "hyper" in SKIP:
                        return hbuf.tile([128, 512], F32R, tag=f"h{r}", name="hn")
                    ps1 = ps_mlp.tile([128, 512], f32, tag="mlp", name="ps1")
                    for g in range(4):
                        nc.tensor.matmul(
                            ps1,
                            fw1_sb[:, g * 128: (g + 1) * 128],
                            fw1_sb[:, 512 + g * 512: 512 + (g + 1) * 512],
                            start=(g == 0), stop=(g == 3),
                        )
                    h = hbuf.tile([128, 512], F32R, tag=f"h{r}a", name="h1")
                    nc.scalar.activation(h, ps1, Relu, bias=bs_sb[:, 0:1], scale=1.0)
                    for i in (2, 3, 4):
                        ps = ps_mlp.tile([128, 512], f32, tag="mlp", name="ps")
                        nc.tensor.matmul(
                            ps, wblk_sb[:, (i - 2) * 128: (i - 1) * 128], h,
                            start=True, stop=True,
                        )
                        h = hbuf.tile([128, 512], F32R,
                                      tag=f"h{r}{'ab'[i % 2]}", name="hn")
                        nc.scalar.activation(
                            h, ps, Relu, bias=bs_sb[:, i - 1: i], scale=1.0
                        )
                    return h

                def kerns(r, h):
                    # rounds over A: 4 row-tiled matmuls (one per class) into a
                    # 4-bank PSUM tile per (A, cc); ACT copy adds the kern bias
                    km_all = {}
                    for g in range(4):
                        for A in range(4):
                            _, beta, _ = meta[g][A]
                            wide = MCOLS if beta == 0 else QC
                            km = kern_pool.tile(
                                [128, 2, QR, wide], bf16, tag=f"k{g}_{A}",
                                name="km",
                            )
                            km_all[(g, A)] = km
                            if beta == 0:
                                border = bass.AP(
                                    tensor=km.tensor,
                                    offset=km.offset,
                                    ap=[km.ap[0], km.ap[1], km.ap[2], [65, 2]],
                                )
                                nc.gpsimd.memset(border, 0.0)
                    if "hyper" in SKIP:
                        return km_all
                    for A in range(4):
                        for cc in range(2):
                            m = A * 2 + cc
                            ps = ps_kern.tile([128, 4, 512], f32, tag="kern_ps",
                                              name="ps")
                            for g in range(4):
                                nc.tensor.matmul(
                                    ps[:, g, :],
                                    wstk_sb[32 * g: 32 * g + 32,
                                            m * 128: (m + 1) * 128],
                                    h[32 * g: 32 * g + 32, :],
                                    start=True, stop=True, skip_group_check=True,
                                    tile_position=(32 * g, 0),
                                )
                            for g in range(4):
                                km = km_all[(g, A)]
                                _, beta, _ = meta[g][A]
                                dst = (km[:, cc, :, 1:65] if beta == 0
                                       else km[:, cc, :, :])
                                for _ in range(2 if "kcopy" in DUP else 1):
                                    nc.scalar.activation(
                                        dst,
                                        ps[:, g, :].rearrange(
                                            "p (a b) -> p a b", a=QR),
                                        Ident, bias=bk_sb[:, m + 8 * g: m + 8 * g + 1],
                                        scale=1.0,
                                    )
                    return km_all

                def apply_cls(r, g, km_all):
                    # products: one batch-broadcast mul per A covering both cc
                    za = {}
                    for A in range(4):
                        km = km_all[(g, A)]
                        alpha, beta, _ = meta[g][A]
                        kb = bass.AP(
                            tensor=km.tensor,
                            offset=km.offset,
                            ap=[km.ap[0], km.ap[1], [0, B], *km.ap[2:]],
                        )
                        rows = slice(1 + alpha, 1 + alpha + QR)
                        if beta == 0:
                            z = zbuf.tile([128, 2, B, QR, MCOLS], bf16,
                                          tag="z", name="z")
                            for _ in range(2 if "prod" in DUP else 1):
                                nc.vector.tensor_mul(
                                    z, x_sb[:, :, :, rows, 0:MCOLS], kb
                                )
                            za[A] = [
                                [z[:, cc, b, :, 1:65] for b in range(B)]
                                for cc in range(2)
                            ]
                        else:
                            c0 = 1 + beta  # 0 or 2, 4B-aligned either way
                            z = zbuf.tile([128, 2, B, QR, QC], bf16,
                                          tag="z", name="z")
                            for _ in range(2 if "prod" in DUP else 1):
                                nc.vector.tensor_mul(
                                    z, x_sb[:, :, :, rows, c0: c0 + QC], kb
                                )
                            za[A] = [
                                [z[:, cc, b] for b in range(B)]
                                for cc in range(2)
                            ]
                    if "apply" in SKIP:
                        return
                    for half in range(2):
                        yp = ps_y.tile([128, 2, 512], f32, tag="y",
                                       name="yp")
                        for rep_mm in range(2 if "ymm" in DUP else 1):
                            for cc in range(2):
                                for A in range(4):
                                    for b2 in range(2):
                                        nc.tensor.matmul(
                                            yp[:, b2, :].rearrange(
                                                "p (a b) -> p a b", a=QR),
                                            wproj_sb[:, cc, :],
                                            za[A][cc][half * 2 + b2],
                                            start=(cc == 0 and A == 0),
                                            stop=(cc == 1 and A == 3),
                                            skip_group_check=True,
                                        )
                        ys = ybuf.tile([F_OUT, 1024], bf16, tag="ysb", name="ys")
                        nc.scalar.activation(
                            ys, yp.rearrange("p a b -> p (a b)"), Ident,
                            bias=bb_sb[:, 0:1], scale=1.0,
                        )
                        nc.sync.dma_start(
                            y_d[:, g, half * 2: half * 2 + 2],
                            ys.rearrange("p (a b) -> p a b", a=2),
                        )

                # software pipeline: all MLPs upfront, kern production one
                # rep ahead of its apply stage
                hs = [mlp(r) for r in reps]
                ks = kerns(reps[0], hs[0])
                for idx, r in enumerate(reps):
                    ks_next = (
                        kerns(reps[idx + 1], hs[idx + 1])
                        if idx + 1 < len(reps) else None
                    )
                    for g in range(4):
                        apply_cls(r, g, ks)
                    ks = ks_next

            if loop_repeat > 1:
                with tc.For_i(
                    0, loop_repeat, 1,
                    hint_engines=(mybir.EngineType.PE, mybir.EngineType.Activation),
                    staggered_reset=staggered,
                ):
                    _body_all()
            else:
                _body_all()

    nc.compile()
    return nc


import os

UNROLL = int(os.environ.get("CUF_UNROLL", "2"))
STAGGERED = bool(int(os.environ.get("CUF_STAG", "0")))
# timing-ablation knobs (local experiments only)
SKIP = set(os.environ.get("CUF_SKIP", "").split(","))
DUP = set(os.environ.get("CUF_DUP", "").split(","))


def get_program(repeat: int = 1, loop_repeat: int = 1):
    # Amortize the For_i back-edge barrier (~2us drain + sem reset) and the
    # per-iteration pipeline fill/drain by unrolling the body inside the
    # hardware loop whenever the requested trip count allows it.
    if repeat == 1 and loop_repeat > 1 and loop_repeat % UNROLL == 0:
        repeat, loop_repeat = UNROLL, loop_repeat // UNROLL
    key = f"nc{repeat}_{loop_repeat}"
    nc = _CACHE.get(key)
    if nc is None:
        nc = _CACHE[key] = _build_program(repeat, loop_repeat, STAGGERED)
    return nc


# --------------------------------------------------------------------- entry
def kernel(**inputs) -> np.ndarray:
    nc = get_program()
    in_maps = _host_prep(inputs)
    res = bass_utils.run_bass_kernel_spmd(
        nc, in_maps, core_ids=list(range(N_CORES))
    )
    return _gather(res.results)
